# revision 1
# baseline (speedup 1.0000x reference)
"""Guided filter (He) on 8 trn2 NeuronCores, batch-parallel.

Per core: one sample [3,1024,1024]. Box filters are separable:
- H-direction (partition axis): TensorE matmuls with three shared 0/1 banded
  matrices (bf16, exact), accumulating fp32 in PSUM.
- W-direction (free axis): DVE tensor_tensor_scan cumsum into a padded buffer
  (31 leading zeros, 31 trailing clamped copies of the total), then a single
  shifted tensor_sub. Zero-padding == border-clipped window sum.
Algebra stays fp32 on DVE/ACT; 1/(var+eps) uses a quadratic fit (denominator
range is [1.3, 1.56], fit error ~2e-4 relative).
"""
import sys
sys.path.insert(0, "/opt/trn_rl_repo")

import numpy as np
import ml_dtypes
from contextlib import ExitStack

B, C, H, W = 8, 3, 1024, 1024
R_RAD = 30
EPS = 1.3
NT = H // 128  # 8 partition tiles per image
F32 = None  # filled after imports
BF16 = None

MAX_WAITS = 1


def _split_excess_waits(nc, mybir):
    """walrus rejects >4 (sometimes >2) sem waits on one instruction; the
    tile-exit drain accumulates one per DMA queue + engine. Move excess waits
    onto same-engine NoOps inserted just before the offending instruction."""
    for fn in nc.m.functions:
        for blk in fn.blocks:
            new_insts, changed = [], False
            for inst in blk.instructions:
                si = inst.sync_info
                if si is not None and len(si.on_wait) > MAX_WAITS:
                    waits = list(si.on_wait)
                    keep = waits[-MAX_WAITS:]
                    rest = waits[:-MAX_WAITS]
                    for ci in range(0, len(rest), MAX_WAITS):
                        nop = mybir.InstNoOp(
                            name=f"{inst.name}-wsplit{ci}", ins=[], outs=[])
                        nop.engine = inst.engine
                        nop.sync_info = mybir.SyncInfo(
                            on_wait=rest[ci:ci + MAX_WAITS], on_update=[])
                        new_insts.append(nop)
                    inst.sync_info = mybir.SyncInfo(
                        on_wait=keep, on_update=list(si.on_update))
                    changed = True
                new_insts.append(inst)
            if changed:
                blk.instructions = new_insts


def _host_constants():
    # banded H-sum matrices: out[j] (tile m) sums input rows of tiles m-1/m/m+1
    k = np.arange(128)[:, None]
    j = np.arange(128)[None, :]
    bA = ((k - j) >= 98).astype(np.float32)          # prev tile: k-128 in [j-30, j+30]
    bB = (np.abs(k - j) <= 30).astype(np.float32)    # same tile
    bC = ((j - k) >= 98).astype(np.float32)          # next tile
    nh = (np.minimum(np.arange(H) + R_RAD, H - 1)
          - np.maximum(np.arange(H) - R_RAD, 0) + 1).astype(np.float32)
    invN = (1.0 / (nh[:, None] * nh[None, :])).astype(np.float32)
    # quadratic fit of 1/(y+EPS) on y in [0, 0.26] (var of [0,1] data <= 0.25)
    ys = np.linspace(0.0, 0.26, 201)
    c2, c1, c0 = np.polyfit(ys, 1.0 / (ys + EPS), 2)
    return (bA.astype(ml_dtypes.bfloat16), bB.astype(ml_dtypes.bfloat16),
            bC.astype(ml_dtypes.bfloat16), invN, float(c2), float(c1), float(c0))


def _build_program(c2, c1, c0):
    import concourse.bass as bass
    import concourse.tile as tile
    from concourse import mybir

    f32, bf16 = mybir.dt.float32, mybir.dt.bfloat16
    ADD, SUB, MULT = (mybir.AluOpType.add, mybir.AluOpType.subtract,
                      mybir.AluOpType.mult)

    nc = bass.Bass("TRN2", debug=False)
    R_d = nc.dram_tensor("R", [C, H, W], f32, kind="ExternalInput").ap()
    I_d = nc.dram_tensor("I", [C, H, W], f32, kind="ExternalInput").ap()
    invN_d = nc.dram_tensor("invN", [H, W], f32, kind="ExternalInput").ap()
    bA_d = nc.dram_tensor("bandA", [128, 128], bf16, kind="ExternalInput").ap()
    bB_d = nc.dram_tensor("bandB", [128, 128], bf16, kind="ExternalInput").ap()
    bC_d = nc.dram_tensor("bandC", [128, 128], bf16, kind="ExternalInput").ap()
    q_d = nc.dram_tensor("q", [C, H, W], f32, kind="ExternalOutput").ap()

    SW = 31 + W + 31  # padded scan buffer width

    with tile.TileContext(nc) as tc, ExitStack() as ctx:
        consts = ctx.enter_context(tc.tile_pool(name="consts", bufs=1))
        inv0 = consts.tile([128, W], f32, tag="invN0")
        nc.sync.dma_start(inv0[:], invN_d[0:128, :])
        invM = consts.tile([128, W], f32, tag="invNM")
        nc.sync.dma_start(invM[:], invN_d[128:256, :])
        inv7 = consts.tile([128, W], f32, tag="invN7")
        nc.sync.dma_start(inv7[:], invN_d[(NT - 1) * 128:NT * 128, :])
        invN_t = [inv0] + [invM] * (NT - 2) + [inv7]
        bA = consts.tile([128, 128], bf16, tag="bA")
        bB = consts.tile([128, 128], bf16, tag="bB")
        bC = consts.tile([128, 128], bf16, tag="bC")
        nc.sync.dma_start(bA[:], bA_d[:, :])
        nc.sync.dma_start(bB[:], bB_d[:, :])
        nc.sync.dma_start(bC[:], bC_d[:, :])
        zeros = consts.tile([128, W], f32, tag="zeros")
        nc.gpsimd.memset(zeros[:], 0.0)
        ones31 = consts.tile([128, 31], f32, tag="ones31")
        nc.gpsimd.memset(ones31[:], 1.0)

        io_pool = ctx.enter_context(tc.tile_pool(name="io", bufs=2))
        img_pool = ctx.enter_context(tc.tile_pool(name="img", bufs=1))
        ab_pool = ctx.enter_context(tc.tile_pool(name="ab", bufs=1))
        psum_pool = ctx.enter_context(tc.tile_pool(name="ps", bufs=4, space="PSUM"))
        scan_pool = ctx.enter_context(tc.tile_pool(name="scan", bufs=2))
        sx_pool = ctx.enter_context(tc.tile_pool(name="sx", bufs=1))
        alg_pool = ctx.enter_context(tc.tile_pool(name="alg", bufs=1))

        def hbox_mm(src_tiles, t, band_set):
            """H-direction banded sum of row-tile t into a fresh PSUM tile."""
            ps = psum_pool.tile([128, W], f32, tag="ps")
            for nch in range(2):
                cols = slice(nch * 512, nch * 512 + 512)
                seq = []
                if t > 0:
                    seq.append((band_set[0], src_tiles[t - 1]))
                seq.append((band_set[1], src_tiles[t]))
                if t < NT - 1:
                    seq.append((band_set[2], src_tiles[t + 1]))
                for i, (bd, rhs) in enumerate(seq):
                    nc.tensor.matmul(ps[:, cols], bd[:], rhs[:, cols],
                                     start=(i == 0), stop=(i == len(seq) - 1))
            return ps

        def wbox(ps, tag):
            """W-direction clipped window sum of a PSUM tile -> SBUF fp32."""
            sb = scan_pool.tile([128, SW], f32, tag="scan")
            nc.gpsimd.memset(sb[:, 0:31], 0.0)
            nc.vector.tensor_tensor_scan(
                sb[:, 31:31 + W], ps[:], zeros[:], 0.0, op0=ADD, op1=ADD)
            nc.scalar.activation(
                sb[:, 31 + W:SW], ones31[:],
                mybir.ActivationFunctionType.Copy, scale=sb[:, 30 + W:31 + W])
            out = sx_pool.tile([128, W], f32, tag=tag)
            nc.vector.tensor_sub(out[:], sb[:, 61:61 + W], sb[:, 0:W])
            return out

        band = (bA, bB, bC)

        for c in range(C):
            Rb, Ib, Pb, Sb_ = [], [], [], []
            for t in range(NT):
                rt = io_pool.tile([128, W], f32, tag="rload")
                nc.sync.dma_start(rt[:], R_d[c, t * 128:(t + 1) * 128, :])
                it = io_pool.tile([128, W], f32, tag="iload")
                nc.sync.dma_start(it[:], I_d[c, t * 128:(t + 1) * 128, :])
                rb = img_pool.tile([128, W], bf16, tag=f"Rb{t}")
                nc.scalar.copy(rb[:], rt[:])
                ib = img_pool.tile([128, W], bf16, tag=f"Ib{t}")
                nc.scalar.copy(ib[:], it[:])
                pb = img_pool.tile([128, W], bf16, tag=f"Pb{t}")
                nc.vector.tensor_mul(pb[:], rb[:], ib[:])
                sb2 = img_pool.tile([128, W], bf16, tag=f"Sq{t}")
                nc.scalar.square(sb2[:], rb[:])
                Rb.append(rb); Ib.append(ib); Pb.append(pb); Sb_.append(sb2)

            ab_t, bb_t = [], []
            for t in range(NT):
                S_R = wbox(hbox_mm(Rb, t, band), "sR")
                S_I = wbox(hbox_mm(Ib, t, band), "sI")
                S_P = wbox(hbox_mm(Pb, t, band), "sP")
                S_S = wbox(hbox_mm(Sb_, t, band), "sS")

                inv = invN_t[t]
                m_R = alg_pool.tile([128, W], f32, tag="g1")
                nc.vector.tensor_mul(m_R[:], S_R[:], inv[:])
                m_I = alg_pool.tile([128, W], f32, tag="g2")
                nc.vector.tensor_mul(m_I[:], S_I[:], inv[:])
                t1 = alg_pool.tile([128, W], f32, tag="g4")
                nc.vector.tensor_mul(t1[:], m_R[:], m_I[:])
                mRI = alg_pool.tile([128, W], f32, tag="g5")
                nc.vector.tensor_mul(mRI[:], S_P[:], inv[:])
                cov = alg_pool.tile([128, W], f32, tag="g3")
                nc.vector.tensor_sub(cov[:], mRI[:], t1[:])
                t2 = alg_pool.tile([128, W], f32, tag="g4b")
                nc.scalar.square(t2[:], m_R[:])
                mRR = alg_pool.tile([128, W], f32, tag="g5b")
                nc.vector.tensor_mul(mRR[:], S_S[:], inv[:])
                var = alg_pool.tile([128, W], f32, tag="g6")
                nc.vector.tensor_sub(var[:], mRR[:], t2[:])
                # rec ~= 1/(var+EPS) quadratic
                u = alg_pool.tile([128, W], f32, tag="g4b")
                nc.vector.tensor_scalar(u[:], var[:], c2, c1, op0=MULT, op1=ADD)
                v = alg_pool.tile([128, W], f32, tag="g5b")
                nc.vector.tensor_mul(v[:], u[:], var[:])
                rec = alg_pool.tile([128, W], f32, tag="g4b")
                nc.vector.tensor_scalar(rec[:], v[:], c0, None, op0=ADD)
                a32 = alg_pool.tile([128, W], f32, tag="g5b")
                nc.vector.tensor_mul(a32[:], cov[:], rec[:])
                ab = ab_pool.tile([128, W], bf16, tag=f"a{t}")
                nc.scalar.copy(ab[:], a32[:])
                t3 = alg_pool.tile([128, W], f32, tag="g6b")
                nc.vector.tensor_mul(t3[:], a32[:], m_R[:])
                bb = ab_pool.tile([128, W], bf16, tag=f"b{t}")
                nc.vector.tensor_sub(bb[:], m_I[:], t3[:])
                ab_t.append(ab); bb_t.append(bb)

            for t in range(NT):
                S_a = wbox(hbox_mm(ab_t, t, band), "sa")
                S_b = wbox(hbox_mm(bb_t, t, band), "sb")
                inv = invN_t[t]
                m_a = alg_pool.tile([128, W], f32, tag="g1")
                nc.vector.tensor_mul(m_a[:], S_a[:], inv[:])
                q1 = alg_pool.tile([128, W], f32, tag="g2")
                nc.vector.tensor_mul(q1[:], m_a[:], Rb[t][:])
                m_b = alg_pool.tile([128, W], f32, tag="g3")
                nc.vector.tensor_mul(m_b[:], S_b[:], inv[:])
                qt = io_pool.tile([128, W], f32, tag="qout")
                nc.vector.tensor_add(qt[:], q1[:], m_b[:])
                nc.sync.dma_start(q_d[c, t * 128:(t + 1) * 128, :], qt[:])

    _split_excess_waits(nc, mybir)
    return nc


_CACHED = {}


def kernel(I, R):
    from concourse import bass_utils

    I = np.asarray(I, dtype=np.float32)
    R = np.asarray(R, dtype=np.float32)
    bA, bB, bC, invN, c2, c1, c0 = _host_constants()
    if "nc" not in _CACHED:
        _CACHED["nc"] = _build_program(c2, c1, c0)
    nc = _CACHED["nc"]
    in_maps = [
        {"R": R[b], "I": I[b], "invN": invN,
         "bandA": bA, "bandB": bB, "bandC": bC}
        for b in range(B)
    ]
    res = bass_utils.run_bass_kernel_spmd(nc, in_maps, core_ids=list(range(B)))
    out = np.stack([np.asarray(res.results[b]["q"]) for b in range(B)], axis=0)
    return out.astype(np.float32)

